# revision 8
# baseline (speedup 1.0000x reference)
"""Trainium2 Bass kernel for CausalSelfAttention (B=4, T=2048, C=2048, H=16).

Sharding: 8 cores = 4 batches x 2 head-groups (8 heads each).
Each core computes q/k/v projections for its heads, RoPE, causal attention,
and a partial output projection (row-parallel c_proj over its heads' columns).
Host sums the two partials per batch (standard row-parallel TP unshard).

v3 structure:
  - bf16 on-chip; host pre-arranges every DRAM operand so each DMA is
    contiguous per partition (fat descriptors; the original layout was
    descriptor-bound at ~23GB/s effective).
  - The next chunk's qkv-projection matmuls are software-pipelined INTO the
    attention stream of the current chunk (the PE queue is in-order, so
    filler matmuls between scores and att@v hide the scalar-engine exp
    latency). The output projection is likewise interleaved into the last
    chunk's attention. Emission order k -> v -> q keeps the q-tile ring
    small.
  - Softmax normalization: per-head ones-matmul row-sum -> DMA to DRAM ->
    per-chunk batched reciprocal on [8, 512] (DVE reciprocal cost scales
    with per-lane elements, so batching heads is 8x cheaper than per-head
    [128,512] reciprocals) -> replicate-DMA broadcast -> in-place gpsimd
    multiply normalizes yT. The whole chain hides under the next chunk's
    merged stream.
"""

import numpy as np
import ml_dtypes

import concourse.bass as bass
import concourse.mybir as mybir
import concourse.tile as tile
from concourse import bacc
from concourse.alu_op_type import AluOpType
from concourse.bass import ds
from concourse.bass_utils import run_bass_kernel_spmd

F16 = ml_dtypes.bfloat16
F32 = np.float32

B = 4
C = 2048
H = 16
D = 128
HPC = 8          # heads per core
P = 128
CH = 512         # tq chunk width
NCT = C // P     # 16 contraction tiles
AF = mybir.ActivationFunctionType
SCALE = 1.0 / float(np.sqrt(np.float32(D)))


def build_nc(T=2048):
    NCH = T // CH
    dt = mybir.dt
    f16 = dt.bfloat16
    nc = bacc.Bacc(None, target_bir_lowering=False)

    xh = nc.dram_tensor("xh", [NCH, P, NCT, CH], f16, kind="ExternalInput")
    wqh = nc.dram_tensor("wqh", [HPC, P, NCT, D], f16, kind="ExternalInput")
    wkh = nc.dram_tensor("wkh", [HPC, P, NCT, D], f16, kind="ExternalInput")
    wvh = nc.dram_tensor("wvh", [2, P, NCT, CH], f16, kind="ExternalInput")
    wph = nc.dram_tensor("wph", [P, HPC, C], f16, kind="ExternalInput")
    aba = nc.dram_tensor("aba", [D, T], f16, kind="ExternalInput")
    abb = nc.dram_tensor("abb", [D, T], f16, kind="ExternalInput")
    bq = nc.dram_tensor("bq", [D, HPC], dt.float32, kind="ExternalInput")
    bk = nc.dram_tensor("bk", [D, HPC], dt.float32, kind="ExternalInput")
    bv = nc.dram_tensor("bv", [1, HPC * D], f16, kind="ExternalInput")
    bp = nc.dram_tensor("bp", [1, C], f16, kind="ExternalInput")
    maskd = nc.dram_tensor("maskd", [P, D], f16, kind="ExternalInput")
    pt = nc.dram_tensor("pt", [D, D], f16, kind="ExternalInput")
    onc = nc.dram_tensor("onc", [P, 1], f16, kind="ExternalInput")
    out = nc.dram_tensor("out", [T // P, C // CH, P, CH], f16, kind="ExternalOutput")
    scr_r = nc.dram_tensor("scr_r", [NCH, HPC, CH], f16)

    with nc.allow_low_precision("bf16 attention pipeline"):
        with (
            tile.TileContext(nc) as tc,
            tc.tile_pool(name="consts", bufs=1) as consts,
            tc.tile_pool(name="keep", bufs=1) as keep,
        ):
            mask_sb = consts.tile([P, D], f16)
            pt_sb = consts.tile([D, D], f16)
            bq_sb = consts.tile([D, HPC], dt.float32)
            bk_sb = consts.tile([D, HPC], dt.float32)
            bv_rep = consts.tile([P, HPC * D], f16)
            bp_rep = consts.tile([P, C], f16)
            onc_sb = consts.tile([P, 1], f16)

            def load_consts_small():
                nc.sync.dma_start(out=mask_sb, in_=maskd[:])
                nc.sync.dma_start(out=pt_sb, in_=pt[:])
                nc.sync.dma_start(out=bq_sb, in_=bq[:])
                nc.sync.dma_start(out=bk_sb, in_=bk[:])
                nc.sync.dma_start(out=onc_sb, in_=onc[:])

            def load_consts_big():
                nc.sync.dma_start(
                    out=bv_rep, in_=bv[0][None, :].to_broadcast([P, HPC * D])
                )
                nc.sync.dma_start(
                    out=bp_rep, in_=bp[0][None, :].to_broadcast([P, C])
                )

            kT = keep.tile([P, HPC, T], f16)
            vS = keep.tile([P, HPC, T], f16)
            yT = keep.tile([P, HPC, T], f16)

            with (
                tc.tile_pool(name="xwp", bufs=1) as xwp,
                tc.tile_pool(name="wtp", bufs=2) as wtp,
                tc.tile_pool(name="wvp", bufs=2) as wvp,
                tc.tile_pool(name="work", bufs=2) as work,
                tc.tile_pool(name="qpp", bufs=10) as qpp,
                tc.tile_pool(name="denp", bufs=2) as denp,
                tc.tile_pool(name="ps_acc", bufs=2, space="PSUM") as ps_acc,
                tc.tile_pool(name="ps_aux", bufs=1, space="PSUM") as ps_aux,
                tc.tile_pool(name="ps_s", bufs=3, space="PSUM") as ps_s,
                tc.tile_pool(name="ps_y", bufs=2, space="PSUM") as ps_y,
            ):
                wps_halves = [None, None]
                qp_lists = {}

                def make_qkv_stream(jj):
                    """Generator emitting chunk jj's qkv projections + RoPE in
                    ~850ns PE quanta (4 matmuls per yield). Order k -> v -> q
                    so q-tile ring reuse lands after the consuming attention
                    heads have finished."""
                    cols2 = ds(jj * CH, CH)
                    a_sl = work.tile([D, CH], f16, tag="abA", bufs=2)
                    nc.sync.dma_start(out=a_sl, in_=aba[:, cols2])
                    b_sl = work.tile([D, CH], f16, tag="abB", bufs=2)
                    nc.sync.dma_start(out=b_sl, in_=abb[:, cols2])
                    xc = xwp.tile([P, NCT, CH], f16, tag="xc")
                    for cg in range(4):
                        nc.sync.dma_start(
                            out=xc[:, ds(cg * 4, 4), :], in_=xh[jj, :, ds(cg * 4, 4)]
                        )
                    qp_lists[jj] = []
                    wt_q = []

                    def dma_wt(qk, h):
                        wt = wtp.tile([P, NCT, D], f16, tag="wt")
                        nc.sync.dma_start(out=wt, in_=(wqh if qk == 0 else wkh)[h])
                        wt_q.append(wt)

                    if jj == 0:
                        load_consts_small()
                    dma_wt(1, 0)  # first k weight prefetched at stream creation

                    def emit_rope(raw, dest):
                        # q'/k' = A (.) raw + B (.) (P @ raw); rotate-half is a
                        # signed permutation applied as one PE matmul
                        rps = ps_aux.tile([P, CH], dt.float32, tag="aux")
                        nc.tensor.matmul(
                            rps, lhsT=pt_sb, rhs=raw, start=True, stop=True
                        )
                        t1 = work.tile([P, CH], f16, tag="t1", bufs=2)
                        nc.gpsimd.tensor_tensor(
                            out=t1, in0=raw, in1=a_sl, op=AluOpType.mult
                        )
                        t2 = work.tile([P, CH], f16, tag="t2", bufs=2)
                        nc.vector.tensor_tensor(
                            out=t2, in0=rps, in1=b_sl, op=AluOpType.mult
                        )
                        nc.vector.tensor_tensor(
                            out=dest, in0=t1, in1=t2, op=AluOpType.add
                        )

                    def qk_section(qk):
                        bsrc = bq_sb if qk == 0 else bk_sb
                        pending = None
                        for h in range(HPC):
                            wt = wt_q.pop(0)
                            if h + 1 < HPC:
                                dma_wt(qk, h + 1)  # prefetch next head
                            ps = ps_acc.tile([P, CH], dt.float32, tag="acc")
                            for cg in range(4):
                                for ct in range(cg * 4, cg * 4 + 4):
                                    nc.tensor.matmul(
                                        ps,
                                        lhsT=wt[:, ct, :],
                                        rhs=xc[:, ct, :],
                                        start=(ct == 0),
                                        stop=(ct == NCT - 1),
                                    )
                                yield
                            raw = work.tile([P, CH], f16, tag="raw", bufs=2)
                            nc.scalar.activation(
                                raw, ps, AF.Identity, bias=bsrc[:, ds(h, 1)]
                            )
                            if qk == 0:
                                dest = qpp.tile([P, CH], f16, tag="qp")
                                qp_lists[jj].append(dest)
                            else:
                                dest = kT[:, h, cols2]
                            if pending is not None:
                                emit_rope(*pending)
                            pending = (raw, dest)
                        emit_rope(*pending)

                    wvts = []
                    for half in range(2):
                        wvt = wvp.tile([P, NCT, CH], f16, tag="wv")
                        nc.sync.dma_start(out=wvt, in_=wvh[half])
                        wvts.append(wvt)

                    def v_section():
                        if jj == 0:
                            load_consts_big()
                        dma_wt(0, 0)  # prefetch first q weight during v work
                        for half in range(2):
                            wvt = wvts[half]
                            for tt in range(4):
                                ps = ps_acc.tile([P, CH], dt.float32, tag="acc")
                                for cg in range(4):
                                    for ct in range(cg * 4, cg * 4 + 4):
                                        nc.tensor.matmul(
                                            ps,
                                            lhsT=xc[:, ct, ds(tt * D, D)],
                                            rhs=wvt[:, ct, :],
                                            start=(ct == 0),
                                            stop=(ct == NCT - 1),
                                        )
                                    yield
                                ti = 4 * jj + tt
                                nc.vector.tensor_tensor(
                                    out=vS[:, ds(half * 4, 4), ds(ti * D, D)],
                                    in0=ps[:, :].rearrange("p (a b) -> p a b", b=D),
                                    in1=bv_rep[:, ds(half * CH, CH)].rearrange(
                                        "p (a b) -> p a b", b=D
                                    ),
                                    op=AluOpType.add,
                                )

                    def gen():
                        yield from qk_section(1)   # k first
                        yield from v_section()
                        yield from qk_section(0)   # q last (ring pressure)
                        if jj == NCH - 1:
                            # prefetch c_proj weights into the idle wv ring
                            for half in range(2):
                                wps = wvp.tile([P, HPC, C // 2], f16, tag="wv")
                                nc.sync.dma_start(
                                    out=wps,
                                    in_=wph[:, :, ds(half * (C // 2), C // 2)],
                                )
                                wps_halves[half] = wps
                    return gen()

                def emit_proj_tile(tt, oc):
                    wps = wps_halves[oc // 2]
                    wcol = ds((oc % 2) * CH, CH)
                    ps = ps_acc.tile([P, CH], dt.float32, tag="acc")
                    for hc in range(HPC):
                        nc.tensor.matmul(
                            ps,
                            lhsT=yT[:, hc, ds(tt * D, D)],
                            rhs=wps[:, hc, wcol],
                            start=(hc == 0),
                            stop=(hc == HPC - 1),
                        )
                    ot = work.tile([P, CH], f16, tag="ot", bufs=2)
                    nc.vector.tensor_tensor(
                        out=ot, in0=ps, in1=bp_rep[:, ds(oc * CH, CH)],
                        op=AluOpType.add,
                    )
                    nc.sync.dma_start(out=out[tt, oc], in_=ot)

                def proj_stream():
                    # last-chunk token tiles (tt 12..15) depend on yT of the
                    # final chunk; they are emitted only in the drain
                    for tt in range(T // P - 4):
                        for oc in range(C // CH):
                            emit_proj_tile(tt, oc)
                            yield
                    for tt in range(T // P - 4, T // P):
                        for oc in range(C // CH):
                            emit_proj_tile(tt, oc)
                            yield

                # chunk 0 qkv runs dense (nothing to overlap with yet)
                for _ in make_qkv_stream(0):
                    pass

                for j in range(NCH):
                    cols = ds(j * CH, CH)
                    is_last = j == NCH - 1
                    filler = proj_stream() if is_last else make_qkv_stream(j + 1)
                    n_units = 48 if is_last else 96
                    ntk = 4 * (j + 1)
                    total_tiles = HPC * ntk
                    warmup = 24 if is_last else 8
                    state = {"alive": True, "credit": 0.0}
                    rate = n_units / max(total_tiles - warmup, 1)

                    def pull():
                        state["credit"] += rate
                        while state["alive"] and state["credit"] >= 1.0:
                            state["credit"] -= 1.0
                            try:
                                next(filler)
                            except StopIteration:
                                state["alive"] = False

                    tile_ctr = 0
                    den8 = denp.tile([HPC, CH], dt.float32, tag="den8", bufs=2)
                    pend_den = []

                    def emit_dsum():
                        den_a, hh = pend_den.pop(0)
                        dsum = ps_y.tile([1, CH], dt.float32, tag="y")
                        nc.tensor.matmul(
                            dsum, lhsT=onc_sb, rhs=den_a, start=True, stop=True
                        )
                        drow = work.tile([1, CH], dt.float32, tag="drow", bufs=2)
                        nc.vector.tensor_copy(out=drow, in_=dsum)
                        # SBUF->SBUF DMA moves the row to partition hh
                        nc.sync.dma_start(out=den8[ds(hh, 1), :], in_=drow)

                    for h in range(HPC):
                        qp = qp_lists[j][h]
                        den_a = denp.tile([P, CH], f16, tag="dena", bufs=2)
                        yps = ps_y.tile([P, CH], dt.float32, tag="y")
                        exq = []  # (ex, i, off) pending y-matmuls
                        for i in range(ntk):
                            sps = ps_s.tile([P, CH], dt.float32, tag="s")
                            m = i - 4 * j
                            off = max(m, 0) * D  # valid tq cols start here
                            w = CH - off
                            nc.tensor.matmul(
                                sps[:, ds(off, w)],
                                lhsT=kT[:, h, ds(i * D, D)],
                                rhs=qp[:, ds(off, w)],
                                start=True,
                                stop=True,
                            )
                            ex = work.tile([P, CH], f16, tag="ex", bufs=5)
                            nc.scalar.activation(
                                ex[:, ds(off, w)], sps[:, ds(off, w)],
                                AF.Exp, scale=SCALE,
                            )
                            if m >= 0:
                                # triangular mask on the diagonal 128-block
                                nc.vector.tensor_tensor(
                                    out=ex[:, ds(off, D)],
                                    in0=ex[:, ds(off, D)],
                                    in1=mask_sb,
                                    op=AluOpType.mult,
                                )
                            if i == 0:
                                nc.vector.tensor_copy(
                                    out=den_a[:, ds(off, w)], in_=ex[:, ds(off, w)]
                                )
                            else:
                                nc.vector.tensor_tensor(
                                    out=den_a[:, ds(off, w)],
                                    in0=den_a[:, ds(off, w)],
                                    in1=ex[:, ds(off, w)],
                                    op=AluOpType.add,
                                )
                            exq.append((ex, i, off))
                            if len(exq) > 2:
                                pex, pi, poff = exq.pop(0)
                                nc.tensor.matmul(
                                    yps[:, ds(poff, CH - poff)],
                                    lhsT=vS[:, h, ds(pi * D, D)],
                                    rhs=pex[:, ds(poff, CH - poff)],
                                    start=(pi == 0),
                                    stop=False,
                                )
                            tile_ctr += 1
                            if i == 2 and pend_den:
                                emit_dsum()
                            if tile_ctr > warmup:
                                pull()
                        while exq:
                            pex, pi, poff = exq.pop(0)
                            nc.tensor.matmul(
                                yps[:, ds(poff, CH - poff)],
                                lhsT=vS[:, h, ds(pi * D, D)],
                                rhs=pex[:, ds(poff, CH - poff)],
                                start=(pi == 0),
                                stop=(not exq),
                            )
                        # unnormalized y straight into yT (normalized in place
                        # later); row-sum of exp goes to DRAM for batching
                        nc.vector.tensor_copy(out=yT[:, h, cols], in_=yps)
                        pend_den.append((den_a, h))
                    while pend_den:
                        emit_dsum()
                    # batched normalization for all 8 heads of this chunk;
                    # latency hides under the merged stream / drain
                    rec8 = denp.tile([HPC, CH], f16, tag="rec8", bufs=1)
                    nc.vector.reciprocal(rec8, den8)
                    nc.sync.dma_start(out=scr_r[j], in_=rec8)
                    for h in range(HPC):
                        rbc = work.tile([P, CH], f16, tag="rbc", bufs=2)
                        nc.sync.dma_start(
                            out=rbc,
                            in_=scr_r[j, h][None, :].to_broadcast([P, CH]),
                        )
                        norm_eng = nc.vector if is_last else nc.gpsimd
                        norm_eng.tensor_tensor(
                            out=yT[:, h, cols],
                            in0=yT[:, h, cols],
                            in1=rbc,
                            op=AluOpType.mult,
                        )
                    # drain leftover filler (incl. last-chunk proj tiles)
                    while state["alive"]:
                        try:
                            next(filler)
                        except StopIteration:
                            state["alive"] = False
    nc.compile()
    return nc


def _rope_tables(T):
    inv_freq = (
        1.0 / (10000.0 ** (np.arange(0, D, 2, dtype=np.float32) / np.float32(D)))
    ).astype(np.float32)
    t = np.arange(T, dtype=np.float32)
    freqs = t[:, None] * inv_freq[None, :]
    emb = np.concatenate((freqs, freqs), axis=-1)
    cos = np.cos(emb).astype(np.float32)
    sin = np.sin(emb).astype(np.float32)
    A = np.ascontiguousarray((cos + sin).T).astype(F16)
    Bt = np.ascontiguousarray((cos - sin).T).astype(F16)
    return A, Bt


def _rot_pt():
    Pm = np.zeros((D, D), dtype=np.float32)
    for d in range(64):
        Pm[d, 2 * d + 1] = -1.0
        Pm[64 + d, 2 * d] = 1.0
    return np.ascontiguousarray(Pm.T).astype(F16)


def _maskd():
    # maskd[p, c] = 0 where tq < tk within a diagonal 128x128 block
    row = np.arange(P)[:, None]
    col = np.arange(D)[None, :]
    return np.where(col < row, 0.0, 1.0).astype(F16)


def make_in_maps(x, w_attn, b_attn, w_proj, b_proj, T=2048):
    A, Bt = _rope_tables(T)
    ptm = _rot_pt()
    mask = _maskd()
    onc = np.ones((P, 1), dtype=F16)
    NCH = T // CH
    in_maps = []
    for core in range(8):
        b, g = core // 2, core % 2
        gs = slice(g * 1024, (g + 1) * 1024)
        bp_eff = b_proj if g == 0 else np.zeros_like(b_proj)

        xT = np.ascontiguousarray(x[b][:T].T)  # [C, T]
        xhm = np.ascontiguousarray(
            xT.reshape(NCT, P, NCH, CH).transpose(2, 1, 0, 3)
        ).astype(F16)

        wq_g = w_attn[gs, :]                       # [1024, 2048]
        wk_g = w_attn[2048:4096][gs, :]
        wv_g = w_attn[4096:6144][gs, :]
        wqhm = np.ascontiguousarray(
            wq_g.reshape(HPC, D, NCT, P).transpose(0, 3, 2, 1)
        ).astype(F16)
        wkhm = np.ascontiguousarray(
            wk_g.reshape(HPC, D, NCT, P).transpose(0, 3, 2, 1)
        ).astype(F16)
        wvhm = np.ascontiguousarray(
            wv_g.reshape(2, CH, NCT, P).transpose(0, 3, 2, 1)
        ).astype(F16)
        wpT = np.ascontiguousarray(w_proj[:, gs].T)  # [1024, 2048]
        wphm = np.ascontiguousarray(
            wpT.reshape(HPC, P, C).transpose(1, 0, 2)
        ).astype(F16)

        in_maps.append(
            {
                "xh": xhm,
                "wqh": wqhm,
                "wkh": wkhm,
                "wvh": wvhm,
                "wph": wphm,
                "aba": A,
                "abb": Bt,
                "bq": np.ascontiguousarray(
                    b_attn[gs].reshape(HPC, D).T
                ).astype(np.float32),
                "bk": np.ascontiguousarray(
                    b_attn[2048:4096][gs].reshape(HPC, D).T
                ).astype(np.float32),
                "bv": b_attn[4096:6144][gs].reshape(1, HPC * D).astype(F16),
                "bp": bp_eff.reshape(1, C).astype(F16),
                "maskd": mask,
                "pt": ptm,
                "onc": onc,
            }
        )
    return in_maps


_NC_CACHE = {}


def run(x, w_attn, b_attn, w_proj, b_proj, trace=False, trace_cores=None):
    T = x.shape[1]
    if T not in _NC_CACHE:
        _NC_CACHE[T] = build_nc(T)
    nc = _NC_CACHE[T]
    in_maps = make_in_maps(
        np.asarray(x, dtype=np.float32),
        np.asarray(w_attn, dtype=np.float32),
        np.asarray(b_attn, dtype=np.float32),
        np.asarray(w_proj, dtype=np.float32),
        np.asarray(b_proj, dtype=np.float32),
        T=T,
    )
    res = run_bass_kernel_spmd(
        nc, in_maps, core_ids=list(range(8)), trace=trace, trace_cores=trace_cores
    )
    out = np.zeros((B, T, C), dtype=np.float32)
    for b in range(B):
        o0 = res.results[2 * b]["out"].astype(np.float32)
        o1 = res.results[2 * b + 1]["out"].astype(np.float32)
        o = o0 + o1  # [T//P, C//CH, P, CH]
        out[b] = o.transpose(0, 2, 1, 3).reshape(T, C)
    return out, res


def kernel(x, w_attn, b_attn, w_proj, b_proj):
    out, _ = run(x, w_attn, b_attn, w_proj, b_proj, trace=False)
    return out


# revision 9
# speedup vs baseline: 1.0332x; 1.0332x over previous
"""Trainium2 Bass kernel for CausalSelfAttention (B=4, T=2048, C=2048, H=16).

Sharding: 8 cores = 4 batches x 2 head-groups (8 heads each).
Each core computes q/k/v projections for its heads, RoPE, causal attention,
and a partial output projection (row-parallel c_proj over its heads' columns).
Host sums the two partials per batch (standard row-parallel TP unshard).

v3 structure:
  - bf16 on-chip; host pre-arranges every DRAM operand so each DMA is
    contiguous per partition (fat descriptors; the original layout was
    descriptor-bound at ~23GB/s effective).
  - The next chunk's qkv-projection matmuls are software-pipelined INTO the
    attention stream of the current chunk (the PE queue is in-order, so
    filler matmuls between scores and att@v hide the scalar-engine exp
    latency). The output projection is likewise interleaved into the last
    chunk's attention. Emission order k -> v -> q keeps the q-tile ring
    small.
  - Softmax normalization: per-head ones-matmul row-sum -> DMA to DRAM ->
    per-chunk batched reciprocal on [8, 512] (DVE reciprocal cost scales
    with per-lane elements, so batching heads is 8x cheaper than per-head
    [128,512] reciprocals) -> replicate-DMA broadcast -> in-place gpsimd
    multiply normalizes yT. The whole chain hides under the next chunk's
    merged stream.
"""

import numpy as np
import ml_dtypes

import concourse.bass as bass
import concourse.mybir as mybir
import concourse.tile as tile
from concourse import bacc
from concourse.alu_op_type import AluOpType
from concourse.bass import ds
from concourse.bass_utils import run_bass_kernel_spmd

F16 = ml_dtypes.bfloat16
F32 = np.float32

B = 4
C = 2048
H = 16
D = 128
HPC = 8          # heads per core
P = 128
CH = 512         # tq chunk width
NCT = C // P     # 16 contraction tiles
AF = mybir.ActivationFunctionType
SCALE = 1.0 / float(np.sqrt(np.float32(D)))


def build_nc(T=2048):
    NCH = T // CH
    dt = mybir.dt
    f16 = dt.bfloat16
    nc = bacc.Bacc(None, target_bir_lowering=False)

    xh = nc.dram_tensor("xh", [NCH, P, NCT, CH], f16, kind="ExternalInput")
    wqh = nc.dram_tensor("wqh", [HPC, P, NCT, D], f16, kind="ExternalInput")
    wkh = nc.dram_tensor("wkh", [HPC, P, NCT, D], f16, kind="ExternalInput")
    wvh = nc.dram_tensor("wvh", [2, P, NCT, CH], f16, kind="ExternalInput")
    wph = nc.dram_tensor("wph", [P, HPC, C], f16, kind="ExternalInput")
    aba = nc.dram_tensor("aba", [D, T], f16, kind="ExternalInput")
    abb = nc.dram_tensor("abb", [D, T], f16, kind="ExternalInput")
    bq = nc.dram_tensor("bq", [D, HPC], dt.float32, kind="ExternalInput")
    bk = nc.dram_tensor("bk", [D, HPC], dt.float32, kind="ExternalInput")
    bv = nc.dram_tensor("bv", [1, HPC * D], f16, kind="ExternalInput")
    bp = nc.dram_tensor("bp", [1, C], f16, kind="ExternalInput")
    maskd = nc.dram_tensor("maskd", [P, D], f16, kind="ExternalInput")
    pt = nc.dram_tensor("pt", [D, D], f16, kind="ExternalInput")
    onc = nc.dram_tensor("onc", [P, 1], f16, kind="ExternalInput")
    out = nc.dram_tensor("out", [T // P, C // CH, P, CH], f16, kind="ExternalOutput")
    scr_r = nc.dram_tensor("scr_r", [NCH, HPC, CH], f16)

    with nc.allow_low_precision("bf16 attention pipeline"):
        with (
            tile.TileContext(nc) as tc,
            tc.tile_pool(name="consts", bufs=1) as consts,
            tc.tile_pool(name="keep", bufs=1) as keep,
        ):
            mask_sb = consts.tile([P, D], f16)
            pt_sb = consts.tile([D, D], f16)
            bq_sb = consts.tile([D, HPC], dt.float32)
            bk_sb = consts.tile([D, HPC], dt.float32)
            bv_rep = consts.tile([P, HPC * D], f16)
            bp_rep = consts.tile([P, C], f16)
            onc_sb = consts.tile([P, 1], f16)

            def load_consts_small():
                nc.sync.dma_start(out=mask_sb, in_=maskd[:])
                nc.sync.dma_start(out=pt_sb, in_=pt[:])
                nc.sync.dma_start(out=bq_sb, in_=bq[:])
                nc.sync.dma_start(out=bk_sb, in_=bk[:])
                nc.sync.dma_start(out=onc_sb, in_=onc[:])

            def load_consts_big():
                nc.sync.dma_start(
                    out=bv_rep, in_=bv[0][None, :].to_broadcast([P, HPC * D])
                )
                nc.sync.dma_start(
                    out=bp_rep, in_=bp[0][None, :].to_broadcast([P, C])
                )

            kT = keep.tile([P, HPC, T], f16)
            vS = keep.tile([P, HPC, T], f16)
            yT = keep.tile([P, HPC, T], f16)

            with (
                tc.tile_pool(name="xwp", bufs=1) as xwp,
                tc.tile_pool(name="wtp", bufs=2) as wtp,
                tc.tile_pool(name="wvp", bufs=2) as wvp,
                tc.tile_pool(name="work", bufs=2) as work,
                tc.tile_pool(name="qpp", bufs=12) as qpp,
                tc.tile_pool(name="denp", bufs=2) as denp,
                tc.tile_pool(name="ps_acc", bufs=2, space="PSUM") as ps_acc,
                tc.tile_pool(name="ps_aux", bufs=1, space="PSUM") as ps_aux,
                tc.tile_pool(name="ps_s", bufs=3, space="PSUM") as ps_s,
                tc.tile_pool(name="ps_y", bufs=2, space="PSUM") as ps_y,
            ):
                wps_halves = [None, None]
                qp_lists = {}

                def make_qkv_stream(jj):
                    """Generator emitting chunk jj's qkv projections + RoPE in
                    ~850ns PE quanta (4 matmuls per yield). Order k -> v -> q
                    so q-tile ring reuse lands after the consuming attention
                    heads have finished."""
                    cols2 = ds(jj * CH, CH)
                    a_sl = work.tile([D, CH], f16, tag="abA", bufs=2)
                    nc.sync.dma_start(out=a_sl, in_=aba[:, cols2])
                    b_sl = work.tile([D, CH], f16, tag="abB", bufs=2)
                    nc.sync.dma_start(out=b_sl, in_=abb[:, cols2])
                    xc = xwp.tile([P, NCT, CH], f16, tag="xc")
                    nx = 16 if jj == 0 else 4
                    for cg in range(nx):
                        w_ = NCT // nx
                        nc.sync.dma_start(
                            out=xc[:, ds(cg * w_, w_), :],
                            in_=xh[jj, :, ds(cg * w_, w_)],
                        )
                    qp_lists[jj] = []
                    wt_q = []

                    def dma_wt(qk, h):
                        wt = wtp.tile([P, NCT, D], f16, tag="wt")
                        nc.sync.dma_start(out=wt, in_=(wqh if qk == 0 else wkh)[h])
                        wt_q.append(wt)

                    if jj == 0:
                        load_consts_small()
                    dma_wt(1, 0)  # first k weight prefetched at stream creation

                    def emit_rope(raw, dest):
                        # q'/k' = A (.) raw + B (.) (P @ raw); rotate-half is a
                        # signed permutation applied as one PE matmul
                        rps = ps_aux.tile([P, CH], dt.float32, tag="aux")
                        nc.tensor.matmul(
                            rps, lhsT=pt_sb, rhs=raw, start=True, stop=True
                        )
                        t1 = work.tile([P, CH], f16, tag="t1", bufs=2)
                        nc.gpsimd.tensor_tensor(
                            out=t1, in0=raw, in1=a_sl, op=AluOpType.mult
                        )
                        t2 = work.tile([P, CH], f16, tag="t2", bufs=2)
                        nc.vector.tensor_tensor(
                            out=t2, in0=rps, in1=b_sl, op=AluOpType.mult
                        )
                        nc.vector.tensor_tensor(
                            out=dest, in0=t1, in1=t2, op=AluOpType.add
                        )

                    def qk_section(qk):
                        bsrc = bq_sb if qk == 0 else bk_sb
                        pending = None
                        for h in range(HPC):
                            wt = wt_q.pop(0)
                            if h + 1 < HPC:
                                dma_wt(qk, h + 1)  # prefetch next head
                            elif qk == 1:
                                dma_wt(0, 0)       # prefetch first q weight
                            ps = ps_acc.tile([P, CH], dt.float32, tag="acc")
                            for cg in range(4):
                                for ct in range(cg * 4, cg * 4 + 4):
                                    nc.tensor.matmul(
                                        ps,
                                        lhsT=wt[:, ct, :],
                                        rhs=xc[:, ct, :],
                                        start=(ct == 0),
                                        stop=(ct == NCT - 1),
                                    )
                                yield
                            raw = work.tile([P, CH], f16, tag="raw", bufs=2)
                            nc.scalar.activation(
                                raw, ps, AF.Identity, bias=bsrc[:, ds(h, 1)]
                            )
                            if qk == 0:
                                dest = qpp.tile([P, CH], f16, tag="qp")
                                qp_lists[jj].append(dest)
                            else:
                                dest = kT[:, h, cols2]
                            if pending is not None:
                                emit_rope(*pending)
                            pending = (raw, dest)
                        emit_rope(*pending)

                    wvts = []
                    for half in range(2):
                        wvt = wvp.tile([P, NCT, CH], f16, tag="wv")
                        nc.sync.dma_start(out=wvt, in_=wvh[half])
                        wvts.append(wvt)

                    def v_section():
                        if jj == 0:
                            load_consts_big()
                        for half in range(2):
                            wvt = wvts[half]
                            for tt in range(4):
                                ps = ps_acc.tile([P, CH], dt.float32, tag="acc")
                                for cg in range(4):
                                    for ct in range(cg * 4, cg * 4 + 4):
                                        nc.tensor.matmul(
                                            ps,
                                            lhsT=xc[:, ct, ds(tt * D, D)],
                                            rhs=wvt[:, ct, :],
                                            start=(ct == 0),
                                            stop=(ct == NCT - 1),
                                        )
                                    yield
                                ti = 4 * jj + tt
                                nc.vector.tensor_tensor(
                                    out=vS[:, ds(half * 4, 4), ds(ti * D, D)],
                                    in0=ps[:, :].rearrange("p (a b) -> p a b", b=D),
                                    in1=bv_rep[:, ds(half * CH, CH)].rearrange(
                                        "p (a b) -> p a b", b=D
                                    ),
                                    op=AluOpType.add,
                                )

                    def gen():
                        yield from qk_section(1)   # k first
                        yield from qk_section(0)   # q second: its RoPE output
                        # must be ready when the next chunk's attention starts
                        yield from v_section()
                        if jj == NCH - 1:
                            # prefetch c_proj weights into the idle wv ring
                            for half in range(2):
                                wps = wvp.tile([P, HPC, C // 2], f16, tag="wv")
                                nc.sync.dma_start(
                                    out=wps,
                                    in_=wph[:, :, ds(half * (C // 2), C // 2)],
                                )
                                wps_halves[half] = wps
                    return gen()

                def emit_proj_tile(tt, oc):
                    wps = wps_halves[oc // 2]
                    wcol = ds((oc % 2) * CH, CH)
                    ps = ps_acc.tile([P, CH], dt.float32, tag="acc")
                    for hc in range(HPC):
                        nc.tensor.matmul(
                            ps,
                            lhsT=yT[:, hc, ds(tt * D, D)],
                            rhs=wps[:, hc, wcol],
                            start=(hc == 0),
                            stop=(hc == HPC - 1),
                        )
                    ot = work.tile([P, CH], f16, tag="ot", bufs=2)
                    nc.vector.tensor_tensor(
                        out=ot, in0=ps, in1=bp_rep[:, ds(oc * CH, CH)],
                        op=AluOpType.add,
                    )
                    nc.sync.dma_start(out=out[tt, oc], in_=ot)

                def proj_stream():
                    # last-chunk token tiles (tt 12..15) depend on yT of the
                    # final chunk; they are emitted only in the drain
                    for tt in range(T // P - 4):
                        for oc in range(C // CH):
                            emit_proj_tile(tt, oc)
                            yield
                    for tt in range(T // P - 4, T // P):
                        for oc in range(C // CH):
                            emit_proj_tile(tt, oc)
                            yield

                # chunk 0 qkv runs dense (nothing to overlap with yet)
                for _ in make_qkv_stream(0):
                    pass

                for j in range(NCH):
                    cols = ds(j * CH, CH)
                    is_last = j == NCH - 1
                    filler = proj_stream() if is_last else make_qkv_stream(j + 1)
                    n_units = 28 if is_last else 96
                    ntk = 4 * (j + 1)
                    total_tiles = HPC * ntk
                    warmup = 24 if is_last else 8
                    state = {"alive": True, "credit": 0.0}
                    rate = n_units / max(total_tiles - warmup, 1)

                    def pull():
                        state["credit"] += rate
                        while state["alive"] and state["credit"] >= 1.0:
                            state["credit"] -= 1.0
                            try:
                                next(filler)
                            except StopIteration:
                                state["alive"] = False

                    tile_ctr = 0
                    den8 = denp.tile([HPC, CH], dt.float32, tag="den8", bufs=2)
                    pend_den = []

                    def emit_dsum():
                        den_a, hh = pend_den.pop(0)
                        dsum = ps_y.tile([1, CH], dt.float32, tag="y")
                        nc.tensor.matmul(
                            dsum, lhsT=onc_sb, rhs=den_a, start=True, stop=True
                        )
                        drow = work.tile([1, CH], dt.float32, tag="drow", bufs=2)
                        nc.vector.tensor_copy(out=drow, in_=dsum)
                        # SBUF->SBUF DMA moves the row to partition hh
                        nc.sync.dma_start(out=den8[ds(hh, 1), :], in_=drow)

                    for h in range(HPC):
                        qp = qp_lists[j][h]
                        den_a = denp.tile([P, CH], f16, tag="dena", bufs=2)
                        yps = ps_y.tile([P, CH], dt.float32, tag="y")
                        exq = []  # (ex, i, off) pending y-matmuls
                        for i in range(ntk):
                            sps = ps_s.tile([P, CH], dt.float32, tag="s")
                            m = i - 4 * j
                            off = max(m, 0) * D  # valid tq cols start here
                            w = CH - off
                            nc.tensor.matmul(
                                sps[:, ds(off, w)],
                                lhsT=kT[:, h, ds(i * D, D)],
                                rhs=qp[:, ds(off, w)],
                                start=True,
                                stop=True,
                            )
                            ex = work.tile([P, CH], f16, tag="ex", bufs=5)
                            nc.scalar.activation(
                                ex[:, ds(off, w)], sps[:, ds(off, w)],
                                AF.Exp, scale=SCALE,
                            )
                            if m >= 0:
                                # triangular mask on the diagonal 128-block
                                nc.vector.tensor_tensor(
                                    out=ex[:, ds(off, D)],
                                    in0=ex[:, ds(off, D)],
                                    in1=mask_sb,
                                    op=AluOpType.mult,
                                )
                            if i == 0:
                                nc.vector.tensor_copy(
                                    out=den_a[:, ds(off, w)], in_=ex[:, ds(off, w)]
                                )
                            else:
                                nc.vector.tensor_tensor(
                                    out=den_a[:, ds(off, w)],
                                    in0=den_a[:, ds(off, w)],
                                    in1=ex[:, ds(off, w)],
                                    op=AluOpType.add,
                                )
                            exq.append((ex, i, off))
                            if len(exq) > 2:
                                pex, pi, poff = exq.pop(0)
                                nc.tensor.matmul(
                                    yps[:, ds(poff, CH - poff)],
                                    lhsT=vS[:, h, ds(pi * D, D)],
                                    rhs=pex[:, ds(poff, CH - poff)],
                                    start=(pi == 0),
                                    stop=False,
                                )
                            tile_ctr += 1
                            if i == 2 and pend_den:
                                emit_dsum()
                            if tile_ctr > warmup:
                                pull()
                        while exq:
                            pex, pi, poff = exq.pop(0)
                            nc.tensor.matmul(
                                yps[:, ds(poff, CH - poff)],
                                lhsT=vS[:, h, ds(pi * D, D)],
                                rhs=pex[:, ds(poff, CH - poff)],
                                start=(pi == 0),
                                stop=(not exq),
                            )
                        # unnormalized y straight into yT (normalized in place
                        # later); row-sum of exp goes to DRAM for batching
                        nc.vector.tensor_copy(out=yT[:, h, cols], in_=yps)
                        pend_den.append((den_a, h))
                    while pend_den:
                        emit_dsum()
                    # batched normalization for all 8 heads of this chunk;
                    # latency hides under the merged stream / drain
                    rec8 = denp.tile([HPC, CH], f16, tag="rec8", bufs=1)
                    nc.vector.reciprocal(rec8, den8)
                    nc.sync.dma_start(out=scr_r[j], in_=rec8)
                    for h in range(HPC):
                        rbc = work.tile([P, CH], f16, tag="rbc", bufs=2)
                        nc.sync.dma_start(
                            out=rbc,
                            in_=scr_r[j, h][None, :].to_broadcast([P, CH]),
                        )
                        norm_eng = nc.vector if is_last else nc.gpsimd
                        norm_eng.tensor_tensor(
                            out=yT[:, h, cols],
                            in0=yT[:, h, cols],
                            in1=rbc,
                            op=AluOpType.mult,
                        )
                    # drain leftover filler (incl. last-chunk proj tiles)
                    while state["alive"]:
                        try:
                            next(filler)
                        except StopIteration:
                            state["alive"] = False
    nc.compile()
    return nc


def _rope_tables(T):
    inv_freq = (
        1.0 / (10000.0 ** (np.arange(0, D, 2, dtype=np.float32) / np.float32(D)))
    ).astype(np.float32)
    t = np.arange(T, dtype=np.float32)
    freqs = t[:, None] * inv_freq[None, :]
    emb = np.concatenate((freqs, freqs), axis=-1)
    cos = np.cos(emb).astype(np.float32)
    sin = np.sin(emb).astype(np.float32)
    A = np.ascontiguousarray((cos + sin).T).astype(F16)
    Bt = np.ascontiguousarray((cos - sin).T).astype(F16)
    return A, Bt


def _rot_pt():
    Pm = np.zeros((D, D), dtype=np.float32)
    for d in range(64):
        Pm[d, 2 * d + 1] = -1.0
        Pm[64 + d, 2 * d] = 1.0
    return np.ascontiguousarray(Pm.T).astype(F16)


def _maskd():
    # maskd[p, c] = 0 where tq < tk within a diagonal 128x128 block
    row = np.arange(P)[:, None]
    col = np.arange(D)[None, :]
    return np.where(col < row, 0.0, 1.0).astype(F16)


def make_in_maps(x, w_attn, b_attn, w_proj, b_proj, T=2048):
    A, Bt = _rope_tables(T)
    ptm = _rot_pt()
    mask = _maskd()
    onc = np.ones((P, 1), dtype=F16)
    NCH = T // CH
    in_maps = []
    for core in range(8):
        b, g = core // 2, core % 2
        gs = slice(g * 1024, (g + 1) * 1024)
        bp_eff = b_proj if g == 0 else np.zeros_like(b_proj)

        xT = np.ascontiguousarray(x[b][:T].T)  # [C, T]
        xhm = np.ascontiguousarray(
            xT.reshape(NCT, P, NCH, CH).transpose(2, 1, 0, 3)
        ).astype(F16)

        wq_g = w_attn[gs, :]                       # [1024, 2048]
        wk_g = w_attn[2048:4096][gs, :]
        wv_g = w_attn[4096:6144][gs, :]
        wqhm = np.ascontiguousarray(
            wq_g.reshape(HPC, D, NCT, P).transpose(0, 3, 2, 1)
        ).astype(F16)
        wkhm = np.ascontiguousarray(
            wk_g.reshape(HPC, D, NCT, P).transpose(0, 3, 2, 1)
        ).astype(F16)
        wvhm = np.ascontiguousarray(
            wv_g.reshape(2, CH, NCT, P).transpose(0, 3, 2, 1)
        ).astype(F16)
        wpT = np.ascontiguousarray(w_proj[:, gs].T)  # [1024, 2048]
        wphm = np.ascontiguousarray(
            wpT.reshape(HPC, P, C).transpose(1, 0, 2)
        ).astype(F16)

        in_maps.append(
            {
                "xh": xhm,
                "wqh": wqhm,
                "wkh": wkhm,
                "wvh": wvhm,
                "wph": wphm,
                "aba": A,
                "abb": Bt,
                "bq": np.ascontiguousarray(
                    b_attn[gs].reshape(HPC, D).T
                ).astype(np.float32),
                "bk": np.ascontiguousarray(
                    b_attn[2048:4096][gs].reshape(HPC, D).T
                ).astype(np.float32),
                "bv": b_attn[4096:6144][gs].reshape(1, HPC * D).astype(F16),
                "bp": bp_eff.reshape(1, C).astype(F16),
                "maskd": mask,
                "pt": ptm,
                "onc": onc,
            }
        )
    return in_maps


_NC_CACHE = {}


def run(x, w_attn, b_attn, w_proj, b_proj, trace=False, trace_cores=None):
    T = x.shape[1]
    if T not in _NC_CACHE:
        _NC_CACHE[T] = build_nc(T)
    nc = _NC_CACHE[T]
    in_maps = make_in_maps(
        np.asarray(x, dtype=np.float32),
        np.asarray(w_attn, dtype=np.float32),
        np.asarray(b_attn, dtype=np.float32),
        np.asarray(w_proj, dtype=np.float32),
        np.asarray(b_proj, dtype=np.float32),
        T=T,
    )
    res = run_bass_kernel_spmd(
        nc, in_maps, core_ids=list(range(8)), trace=trace, trace_cores=trace_cores
    )
    out = np.zeros((B, T, C), dtype=np.float32)
    for b in range(B):
        o0 = res.results[2 * b]["out"].astype(np.float32)
        o1 = res.results[2 * b + 1]["out"].astype(np.float32)
        o = o0 + o1  # [T//P, C//CH, P, CH]
        out[b] = o.transpose(0, 2, 1, 3).reshape(T, C)
    return out, res


def kernel(x, w_attn, b_attn, w_proj, b_proj):
    out, _ = run(x, w_attn, b_attn, w_proj, b_proj, trace=False)
    return out


# revision 12
# speedup vs baseline: 1.0409x; 1.0075x over previous
"""Trainium2 Bass kernel for CausalSelfAttention (B=4, T=2048, C=2048, H=16).

Sharding: 8 cores = 4 batches x 2 head-groups (8 heads each).
Each core computes q/k/v projections for its heads, RoPE, causal attention,
and a partial output projection (row-parallel c_proj over its heads' columns).
Host sums the two partials per batch (standard row-parallel TP unshard).

v3 structure:
  - bf16 on-chip; host pre-arranges every DRAM operand so each DMA is
    contiguous per partition (fat descriptors; the original layout was
    descriptor-bound at ~23GB/s effective).
  - The next chunk's qkv-projection matmuls are software-pipelined INTO the
    attention stream of the current chunk (the PE queue is in-order, so
    filler matmuls between scores and att@v hide the scalar-engine exp
    latency). The output projection is likewise interleaved into the last
    chunk's attention. Emission order k -> v -> q keeps the q-tile ring
    small.
  - Softmax normalization: per-head ones-matmul row-sum -> DMA to DRAM ->
    per-chunk batched reciprocal on [8, 512] (DVE reciprocal cost scales
    with per-lane elements, so batching heads is 8x cheaper than per-head
    [128,512] reciprocals) -> replicate-DMA broadcast -> in-place gpsimd
    multiply normalizes yT. The whole chain hides under the next chunk's
    merged stream.
"""

import numpy as np
import ml_dtypes

import concourse.bass as bass
import concourse.mybir as mybir
import concourse.tile as tile
from concourse import bacc
from concourse.alu_op_type import AluOpType
from concourse.bass import ds
from concourse.bass_utils import run_bass_kernel_spmd

F16 = ml_dtypes.bfloat16
F32 = np.float32

B = 4
C = 2048
H = 16
D = 128
HPC = 8          # heads per core
P = 128
CH = 512         # tq chunk width
NCT = C // P     # 16 contraction tiles
AF = mybir.ActivationFunctionType
SCALE = 1.0 / float(np.sqrt(np.float32(D)))


def build_nc(T=2048):
    NCH = T // CH
    dt = mybir.dt
    f16 = dt.bfloat16
    nc = bacc.Bacc(None, target_bir_lowering=False)

    xh = nc.dram_tensor("xh", [NCH, P, NCT, CH], f16, kind="ExternalInput")
    wqh = nc.dram_tensor("wqh", [HPC, P, NCT, D], f16, kind="ExternalInput")
    wkh = nc.dram_tensor("wkh", [HPC, P, NCT, D], f16, kind="ExternalInput")
    wvh = nc.dram_tensor("wvh", [2, P, NCT, CH], f16, kind="ExternalInput")
    wph = nc.dram_tensor("wph", [P, HPC, C], f16, kind="ExternalInput")
    aba = nc.dram_tensor("aba", [D, T], f16, kind="ExternalInput")
    abb = nc.dram_tensor("abb", [D, T], f16, kind="ExternalInput")
    bq = nc.dram_tensor("bq", [D, HPC], dt.float32, kind="ExternalInput")
    bk = nc.dram_tensor("bk", [D, HPC], dt.float32, kind="ExternalInput")
    bv = nc.dram_tensor("bv", [1, HPC * D], f16, kind="ExternalInput")
    bp = nc.dram_tensor("bp", [1, C], f16, kind="ExternalInput")
    maskd = nc.dram_tensor("maskd", [P, D], f16, kind="ExternalInput")
    pt = nc.dram_tensor("pt", [D, D], f16, kind="ExternalInput")
    onc = nc.dram_tensor("onc", [P, 1], f16, kind="ExternalInput")
    out = nc.dram_tensor("out", [T // P, C // CH, P, CH], f16, kind="ExternalOutput")
    scr_r = nc.dram_tensor("scr_r", [NCH, HPC, CH], f16)

    with nc.allow_low_precision("bf16 attention pipeline"):
        with (
            tile.TileContext(nc) as tc,
            tc.tile_pool(name="consts", bufs=1) as consts,
            tc.tile_pool(name="keep", bufs=1) as keep,
        ):
            mask_sb = consts.tile([P, D], f16)
            pt_sb = consts.tile([D, D], f16)
            bq_sb = consts.tile([D, HPC], dt.float32)
            bk_sb = consts.tile([D, HPC], dt.float32)
            bv_rep = consts.tile([P, HPC * D], f16)
            bp_rep = consts.tile([P, C], f16)
            onc_sb = consts.tile([P, 1], f16)

            def load_consts_small():
                nc.sync.dma_start(out=mask_sb, in_=maskd[:])
                nc.sync.dma_start(out=pt_sb, in_=pt[:])
                nc.sync.dma_start(out=bq_sb, in_=bq[:])
                nc.sync.dma_start(out=bk_sb, in_=bk[:])
                nc.sync.dma_start(out=onc_sb, in_=onc[:])

            def load_consts_big():
                nc.sync.dma_start(
                    out=bv_rep, in_=bv[0][None, :].to_broadcast([P, HPC * D])
                )
                nc.sync.dma_start(
                    out=bp_rep, in_=bp[0][None, :].to_broadcast([P, C])
                )

            kT = keep.tile([P, HPC, T], f16)
            vS = keep.tile([P, HPC, T], f16)
            yT = keep.tile([P, HPC, T], f16)

            with (
                tc.tile_pool(name="xwp", bufs=1) as xwp,
                tc.tile_pool(name="wtp", bufs=2) as wtp,
                tc.tile_pool(name="wvp", bufs=2) as wvp,
                tc.tile_pool(name="work", bufs=2) as work,
                tc.tile_pool(name="qpp", bufs=12) as qpp,
                tc.tile_pool(name="denp", bufs=2) as denp,
                tc.tile_pool(name="ps_acc", bufs=2, space="PSUM") as ps_acc,
                tc.tile_pool(name="ps_aux", bufs=1, space="PSUM") as ps_aux,
                tc.tile_pool(name="ps_s", bufs=3, space="PSUM") as ps_s,
                tc.tile_pool(name="ps_y", bufs=2, space="PSUM") as ps_y,
            ):
                wps_halves = [None, None]
                qp_lists = {}

                def make_qkv_stream(jj):
                    """Generator emitting chunk jj's qkv projections + RoPE in
                    ~850ns PE quanta (4 matmuls per yield). Order k -> v -> q
                    so q-tile ring reuse lands after the consuming attention
                    heads have finished."""
                    cols2 = ds(jj * CH, CH)
                    a_sl = work.tile([D, CH], f16, tag="abA", bufs=2)
                    nc.sync.dma_start(out=a_sl, in_=aba[:, cols2])
                    b_sl = work.tile([D, CH], f16, tag="abB", bufs=2)
                    nc.sync.dma_start(out=b_sl, in_=abb[:, cols2])
                    xc = xwp.tile([P, NCT, CH], f16, tag="xc")
                    for cg in range(4):
                        nc.sync.dma_start(
                            out=xc[:, ds(cg * 4, 4), :], in_=xh[jj, :, ds(cg * 4, 4)]
                        )
                    qp_lists[jj] = []
                    wt_q = []

                    def dma_wt(qk, h):
                        wt = wtp.tile([P, NCT, D], f16, tag="wt")
                        nc.sync.dma_start(out=wt, in_=(wqh if qk == 0 else wkh)[h])
                        wt_q.append(wt)

                    if jj == 0:
                        load_consts_small()
                    dma_wt(1, 0)  # first k weight prefetched at stream creation

                    def emit_rope(raw, dest):
                        # q'/k' = A (.) raw + B (.) (P @ raw); rotate-half is a
                        # signed permutation applied as one PE matmul
                        rps = ps_aux.tile([P, CH], dt.float32, tag="aux")
                        nc.tensor.matmul(
                            rps, lhsT=pt_sb, rhs=raw, start=True, stop=True
                        )
                        t1 = work.tile([P, CH], f16, tag="t1", bufs=2)
                        nc.gpsimd.tensor_tensor(
                            out=t1, in0=raw, in1=a_sl, op=AluOpType.mult
                        )
                        t2 = work.tile([P, CH], f16, tag="t2", bufs=2)
                        nc.vector.tensor_tensor(
                            out=t2, in0=rps, in1=b_sl, op=AluOpType.mult
                        )
                        nc.vector.tensor_tensor(
                            out=dest, in0=t1, in1=t2, op=AluOpType.add
                        )

                    def qk_section(qk):
                        bsrc = bq_sb if qk == 0 else bk_sb
                        pending = None
                        for h in range(HPC):
                            wt = wt_q.pop(0)
                            if h + 1 < HPC:
                                dma_wt(qk, h + 1)  # prefetch next head
                            elif qk == 1:
                                dma_wt(0, 0)       # prefetch first q weight
                            if qk == 0 and h == 1:
                                dma_wv()           # v weights one section out
                            ps = ps_acc.tile([P, CH], dt.float32, tag="acc")
                            for cg in range(4):
                                for ct in range(cg * 4, cg * 4 + 4):
                                    nc.tensor.matmul(
                                        ps,
                                        lhsT=wt[:, ct, :],
                                        rhs=xc[:, ct, :],
                                        start=(ct == 0),
                                        stop=(ct == NCT - 1),
                                    )
                                yield
                            raw = work.tile([P, CH], f16, tag="raw", bufs=2)
                            nc.scalar.activation(
                                raw, ps, AF.Identity, bias=bsrc[:, ds(h, 1)]
                            )
                            if qk == 0:
                                dest = qpp.tile([P, CH], f16, tag="qp")
                                qp_lists[jj].append(dest)
                            else:
                                dest = kT[:, h, cols2]
                            if pending is not None:
                                emit_rope(*pending)
                            pending = (raw, dest)
                        emit_rope(*pending)

                    wvts = []

                    def dma_wv():
                        for half in range(2):
                            wvt = wvp.tile([P, NCT, CH], f16, tag="wv")
                            nc.sync.dma_start(out=wvt, in_=wvh[half])
                            wvts.append(wvt)

                    def v_section():
                        if jj == 0:
                            load_consts_big()
                        for half in range(2):
                            wvt = wvts[half]
                            for tt in range(4):
                                ps = ps_acc.tile([P, CH], dt.float32, tag="acc")
                                for cg in range(4):
                                    for ct in range(cg * 4, cg * 4 + 4):
                                        nc.tensor.matmul(
                                            ps,
                                            lhsT=xc[:, ct, ds(tt * D, D)],
                                            rhs=wvt[:, ct, :],
                                            start=(ct == 0),
                                            stop=(ct == NCT - 1),
                                        )
                                    yield
                                ti = 4 * jj + tt
                                nc.vector.tensor_tensor(
                                    out=vS[:, ds(half * 4, 4), ds(ti * D, D)],
                                    in0=ps[:, :].rearrange("p (a b) -> p a b", b=D),
                                    in1=bv_rep[:, ds(half * CH, CH)].rearrange(
                                        "p (a b) -> p a b", b=D
                                    ),
                                    op=AluOpType.add,
                                )

                    def gen():
                        yield from qk_section(1)   # k first
                        yield from qk_section(0)   # q second: its RoPE output
                        # must be ready when the next chunk's attention starts
                        yield from v_section()
                        if jj == NCH - 1:
                            # prefetch c_proj weights into the idle wv ring
                            for half in range(2):
                                wps = wvp.tile([P, HPC, C // 2], f16, tag="wv")
                                nc.sync.dma_start(
                                    out=wps,
                                    in_=wph[:, :, ds(half * (C // 2), C // 2)],
                                )
                                wps_halves[half] = wps
                    return gen()

                def emit_proj_tile(tt, oc):
                    wps = wps_halves[oc // 2]
                    wcol = ds((oc % 2) * CH, CH)
                    ps = ps_acc.tile([P, CH], dt.float32, tag="acc")
                    for hc in range(HPC):
                        nc.tensor.matmul(
                            ps,
                            lhsT=yT[:, hc, ds(tt * D, D)],
                            rhs=wps[:, hc, wcol],
                            start=(hc == 0),
                            stop=(hc == HPC - 1),
                        )
                    ot = work.tile([P, CH], f16, tag="ot", bufs=2)
                    nc.vector.tensor_tensor(
                        out=ot, in0=ps, in1=bp_rep[:, ds(oc * CH, CH)],
                        op=AluOpType.add,
                    )
                    nc.sync.dma_start(out=out[tt, oc], in_=ot)

                def proj_stream():
                    # last-chunk token tiles (tt 12..15) depend on yT of the
                    # final chunk; they are emitted only in the drain
                    for tt in range(T // P - 4):
                        for oc in range(C // CH):
                            emit_proj_tile(tt, oc)
                            yield
                    for tt in range(T // P - 4, T // P):
                        for oc in range(C // CH):
                            emit_proj_tile(tt, oc)
                            yield

                # chunk 0 qkv runs dense (nothing to overlap with yet)
                for _ in make_qkv_stream(0):
                    pass

                pend_batch = []

                def emit_norm_batch():
                    jn, den8n, colsn, lastn = pend_batch.pop(0)
                    rec8 = denp.tile([HPC, CH], f16, tag="rec8", bufs=1)
                    nc.vector.reciprocal(rec8, den8n)
                    nc.sync.dma_start(out=scr_r[jn], in_=rec8)
                    for hh in range(HPC):
                        rbc = work.tile([P, CH], f16, tag="rbc", bufs=2)
                        nc.sync.dma_start(
                            out=rbc,
                            in_=scr_r[jn, hh][None, :].to_broadcast([P, CH]),
                        )
                        norm_eng = nc.vector if lastn else nc.gpsimd
                        norm_eng.tensor_tensor(
                            out=yT[:, hh, colsn],
                            in0=yT[:, hh, colsn],
                            in1=rbc,
                            op=AluOpType.mult,
                        )

                for j in range(NCH):
                    cols = ds(j * CH, CH)
                    is_last = j == NCH - 1
                    filler = proj_stream() if is_last else make_qkv_stream(j + 1)
                    n_units = 28 if is_last else 96
                    ntk = 4 * (j + 1)
                    total_tiles = HPC * ntk
                    warmup = 24 if is_last else 8
                    state = {"alive": True, "credit": 0.0}
                    rate = n_units / max(total_tiles - warmup, 1)

                    def pull():
                        state["credit"] += rate
                        while state["alive"] and state["credit"] >= 1.0:
                            state["credit"] -= 1.0
                            try:
                                next(filler)
                            except StopIteration:
                                state["alive"] = False

                    tile_ctr = 0
                    den8 = denp.tile([HPC, CH], dt.float32, tag="den8", bufs=2)
                    pend_den = []

                    def emit_dsum():
                        den_a, hh = pend_den.pop(0)
                        dsum = ps_y.tile([1, CH], dt.float32, tag="y")
                        nc.tensor.matmul(
                            dsum, lhsT=onc_sb, rhs=den_a, start=True, stop=True
                        )
                        drow = work.tile([1, CH], dt.float32, tag="drow", bufs=2)
                        nc.vector.tensor_copy(out=drow, in_=dsum)
                        # SBUF->SBUF DMA moves the row to partition hh
                        nc.sync.dma_start(out=den8[ds(hh, 1), :], in_=drow)

                    for h in range(HPC):
                        qp = qp_lists[j][h]
                        den_a = denp.tile([P, CH], f16, tag="dena", bufs=2)
                        yps = ps_y.tile([P, CH], dt.float32, tag="y")
                        exq = []  # (ex, i, off) pending y-matmuls
                        for i in range(ntk):
                            sps = ps_s.tile([P, CH], dt.float32, tag="s")
                            m = i - 4 * j
                            off = max(m, 0) * D  # valid tq cols start here
                            w = CH - off
                            nc.tensor.matmul(
                                sps[:, ds(off, w)],
                                lhsT=kT[:, h, ds(i * D, D)],
                                rhs=qp[:, ds(off, w)],
                                start=True,
                                stop=True,
                            )
                            ex = work.tile([P, CH], f16, tag="ex", bufs=5)
                            nc.scalar.activation(
                                ex[:, ds(off, w)], sps[:, ds(off, w)],
                                AF.Exp, scale=SCALE,
                            )
                            if m >= 0:
                                # triangular mask on the diagonal 128-block
                                nc.vector.tensor_tensor(
                                    out=ex[:, ds(off, D)],
                                    in0=ex[:, ds(off, D)],
                                    in1=mask_sb,
                                    op=AluOpType.mult,
                                )
                            if i == 0:
                                nc.vector.tensor_copy(
                                    out=den_a[:, ds(off, w)], in_=ex[:, ds(off, w)]
                                )
                            else:
                                nc.vector.tensor_tensor(
                                    out=den_a[:, ds(off, w)],
                                    in0=den_a[:, ds(off, w)],
                                    in1=ex[:, ds(off, w)],
                                    op=AluOpType.add,
                                )
                            exq.append((ex, i, off))
                            if len(exq) > 2:
                                pex, pi, poff = exq.pop(0)
                                nc.tensor.matmul(
                                    yps[:, ds(poff, CH - poff)],
                                    lhsT=vS[:, h, ds(pi * D, D)],
                                    rhs=pex[:, ds(poff, CH - poff)],
                                    start=(pi == 0),
                                    stop=False,
                                )
                            tile_ctr += 1
                            if i == 2 and pend_den:
                                emit_dsum()
                            if tile_ctr > warmup:
                                pull()
                        while exq:
                            pex, pi, poff = exq.pop(0)
                            nc.tensor.matmul(
                                yps[:, ds(poff, CH - poff)],
                                lhsT=vS[:, h, ds(pi * D, D)],
                                rhs=pex[:, ds(poff, CH - poff)],
                                start=(pi == 0),
                                stop=(not exq),
                            )
                        # unnormalized y straight into yT (normalized in place
                        # later); row-sum of exp goes to DRAM for batching
                        nc.vector.tensor_copy(out=yT[:, h, cols], in_=yps)
                        pend_den.append((den_a, h))
                    while pend_den:
                        emit_dsum()
                    # batched normalization for all 8 heads of this chunk is
                    # deferred into the next chunk's attention stream (or
                    # emitted now for the last chunk) so its replicate-DMA
                    # burst stays clear of the next chunk's weight stream
                    pend_batch.append((j, den8, cols, is_last))
                    emit_norm_batch()
                    # drain leftover filler (incl. last-chunk proj tiles)
                    while state["alive"]:
                        try:
                            next(filler)
                        except StopIteration:
                            state["alive"] = False
    nc.compile()
    return nc


def _rope_tables(T):
    inv_freq = (
        1.0 / (10000.0 ** (np.arange(0, D, 2, dtype=np.float32) / np.float32(D)))
    ).astype(np.float32)
    t = np.arange(T, dtype=np.float32)
    freqs = t[:, None] * inv_freq[None, :]
    emb = np.concatenate((freqs, freqs), axis=-1)
    cos = np.cos(emb).astype(np.float32)
    sin = np.sin(emb).astype(np.float32)
    A = np.ascontiguousarray((cos + sin).T).astype(F16)
    Bt = np.ascontiguousarray((cos - sin).T).astype(F16)
    return A, Bt


def _rot_pt():
    Pm = np.zeros((D, D), dtype=np.float32)
    for d in range(64):
        Pm[d, 2 * d + 1] = -1.0
        Pm[64 + d, 2 * d] = 1.0
    return np.ascontiguousarray(Pm.T).astype(F16)


def _maskd():
    # maskd[p, c] = 0 where tq < tk within a diagonal 128x128 block
    row = np.arange(P)[:, None]
    col = np.arange(D)[None, :]
    return np.where(col < row, 0.0, 1.0).astype(F16)


def make_in_maps(x, w_attn, b_attn, w_proj, b_proj, T=2048):
    A, Bt = _rope_tables(T)
    ptm = _rot_pt()
    mask = _maskd()
    onc = np.ones((P, 1), dtype=F16)
    NCH = T // CH
    in_maps = []
    for core in range(8):
        b, g = core // 2, core % 2
        gs = slice(g * 1024, (g + 1) * 1024)
        bp_eff = b_proj if g == 0 else np.zeros_like(b_proj)

        xT = np.ascontiguousarray(x[b][:T].T)  # [C, T]
        xhm = np.ascontiguousarray(
            xT.reshape(NCT, P, NCH, CH).transpose(2, 1, 0, 3)
        ).astype(F16)

        wq_g = w_attn[gs, :]                       # [1024, 2048]
        wk_g = w_attn[2048:4096][gs, :]
        wv_g = w_attn[4096:6144][gs, :]
        wqhm = np.ascontiguousarray(
            wq_g.reshape(HPC, D, NCT, P).transpose(0, 3, 2, 1)
        ).astype(F16)
        wkhm = np.ascontiguousarray(
            wk_g.reshape(HPC, D, NCT, P).transpose(0, 3, 2, 1)
        ).astype(F16)
        wvhm = np.ascontiguousarray(
            wv_g.reshape(2, CH, NCT, P).transpose(0, 3, 2, 1)
        ).astype(F16)
        wpT = np.ascontiguousarray(w_proj[:, gs].T)  # [1024, 2048]
        wphm = np.ascontiguousarray(
            wpT.reshape(HPC, P, C).transpose(1, 0, 2)
        ).astype(F16)

        in_maps.append(
            {
                "xh": xhm,
                "wqh": wqhm,
                "wkh": wkhm,
                "wvh": wvhm,
                "wph": wphm,
                "aba": A,
                "abb": Bt,
                "bq": np.ascontiguousarray(
                    b_attn[gs].reshape(HPC, D).T
                ).astype(np.float32),
                "bk": np.ascontiguousarray(
                    b_attn[2048:4096][gs].reshape(HPC, D).T
                ).astype(np.float32),
                "bv": b_attn[4096:6144][gs].reshape(1, HPC * D).astype(F16),
                "bp": bp_eff.reshape(1, C).astype(F16),
                "maskd": mask,
                "pt": ptm,
                "onc": onc,
            }
        )
    return in_maps


_NC_CACHE = {}


def run(x, w_attn, b_attn, w_proj, b_proj, trace=False, trace_cores=None):
    T = x.shape[1]
    if T not in _NC_CACHE:
        _NC_CACHE[T] = build_nc(T)
    nc = _NC_CACHE[T]
    in_maps = make_in_maps(
        np.asarray(x, dtype=np.float32),
        np.asarray(w_attn, dtype=np.float32),
        np.asarray(b_attn, dtype=np.float32),
        np.asarray(w_proj, dtype=np.float32),
        np.asarray(b_proj, dtype=np.float32),
        T=T,
    )
    res = run_bass_kernel_spmd(
        nc, in_maps, core_ids=list(range(8)), trace=trace, trace_cores=trace_cores
    )
    out = np.zeros((B, T, C), dtype=np.float32)
    for b in range(B):
        o0 = res.results[2 * b]["out"].astype(np.float32)
        o1 = res.results[2 * b + 1]["out"].astype(np.float32)
        o = o0 + o1  # [T//P, C//CH, P, CH]
        out[b] = o.transpose(0, 2, 1, 3).reshape(T, C)
    return out, res


def kernel(x, w_attn, b_attn, w_proj, b_proj):
    out, _ = run(x, w_attn, b_attn, w_proj, b_proj, trace=False)
    return out
